# revision 1
# baseline (speedup 1.0000x reference)
"""Trainium2 Bass kernel for nn_MeanAggregator (time-decayed GNN mean aggregation).

Contract: kernel(**inputs) takes the FULL inputs
  nodes [50000] int, neigh_ids [50000,32] int, neigh_times [50000,32] f32,
  features [500000,128] f32
and returns the FULL output [50000,128] f32.

Strategy: data-parallel shard of the batch dim across 8 NeuronCores, feature
table replicated per core. Per 128-row tile, one indirect (gather) DMA pulls
self+neighbor feature rows into SBUF; the time-decay weights are computed in
one batched phase; the weighted sum runs on DVE (scalar_tensor_tensor chain)
with a slice of neighbor slots offloaded to ScalarE(diag build)+TensorE(PSUM
accumulation).
"""
import os
import sys
import types

import numpy as np

# If a caller sets BASS_TRACE without providing antenv.axon_hooks,
# concourse.bass_utils would crash on import; give it a no-op hook module.
try:
    import antenv.axon_hooks  # noqa: F401
except Exception:
    try:
        import antenv
        _mod = types.ModuleType("antenv.axon_hooks")
        _holder = {"v": None}
        _mod.set_axon_ntff_profile_hook = lambda h: _holder.__setitem__("v", h)
        _mod.get_axon_ntff_profile_hook = lambda: _holder["v"]
        sys.modules["antenv.axon_hooks"] = _mod
        antenv.axon_hooks = _mod
    except Exception:
        pass

import concourse.bacc as bacc
import concourse.mybir as mybir
import concourse.tile as tile
from concourse.bass import IndirectOffsetOnAxis
from concourse.bass_utils import run_bass_kernel_spmd
from concourse.mybir import ActivationFunctionType as act
from concourse.mybir import AluOpType as alu

TIME = 100.0
TAU = 100.0
P = 128
F32 = mybir.dt.float32

N_CORES = 8
B = 50000
K = 32
N = 500000
D = 128
B_CORE = B // N_CORES          # 6250
T = -(-B_CORE // P)            # 49 tiles
B_PAD = T * P                  # 6272
ACT_KS = 10                    # neighbor slots routed via ScalarE+TensorE

LAST_RESULT = None


def _build_kernel(tc, outs, ins, n_tiles, act_ks):
    nc = tc.nc
    feats = ins["features"]
    idx = ins["idx"]
    times = ins["times"]
    out = outs["out"]
    KP1 = K + 1
    Tn = n_tiles

    with (
        tc.tile_pool(name="const", bufs=1) as cpool,
        tc.tile_pool(name="gather", bufs=8) as gpool,
        tc.tile_pool(name="wbig", bufs=1) as wpool,
        tc.tile_pool(name="accs", bufs=3) as apool,
        tc.tile_pool(name="diags", bufs=4) as dpool,
        tc.tile_pool(name="outs", bufs=3) as opool,
    ):
        idx_sb = cpool.tile([P, Tn * KP1], mybir.dt.int32, tag="idx")
        nc.sync.dma_start(idx_sb[:], idx[:, :])
        times_sb = cpool.tile([P, Tn * K], F32, tag="times")
        nc.sync.dma_start(times_sb[:], times[:, :])
        neg1 = cpool.tile([P, 1], F32, tag="neg1")
        nc.vector.memset(neg1[:], -1.0)

        # ---- batched weights phase over all tiles ----
        TK = Tn * K
        e_all = wpool.tile([P, TK], F32, tag="e_all")
        nc.scalar.activation(e_all[:], times_sb[:], act.Exp,
                             bias=neg1[:, :], scale=1.0 / TAU)
        mask = wpool.tile([P, TK], F32, tag="mask")
        nc.vector.tensor_scalar(mask[:], times_sb[:], TIME, None, alu.is_le)
        w_all = wpool.tile([P, TK], F32, tag="w_all")
        nc.vector.tensor_tensor(w_all[:], e_all[:], mask[:], alu.mult)

        total = wpool.tile([P, Tn], F32, tag="total")
        nc.vector.tensor_reduce(
            total[:], w_all[:].rearrange("p (t k) -> p t k", k=K),
            axis=mybir.AxisListType.X, op=alu.add)
        iszero = wpool.tile([P, Tn], F32, tag="iszero")
        nc.vector.tensor_scalar(iszero[:], total[:], 0.0, None, alu.is_equal)
        total_adj = wpool.tile([P, Tn], F32, tag="total_adj")
        nc.vector.tensor_tensor(total_adj[:], total[:], iszero[:], alu.add)
        inv_total = wpool.tile([P, Tn], F32, tag="inv_total")
        nc.vector.reciprocal(inv_total[:], total_adj[:])

        wn_all = wpool.tile([P, TK], F32, tag="wn_all")
        nc.vector.tensor_tensor(
            wn_all[:].rearrange("p (t k) -> p t k", k=K),
            w_all[:].rearrange("p (t k) -> p t k", k=K),
            inv_total[:].to_broadcast((P, Tn, K)),
            alu.mult)
        wnsum = wpool.tile([P, Tn], F32, tag="wnsum")
        nc.vector.tensor_reduce(
            wnsum[:], wn_all[:].rearrange("p (t k) -> p t k", k=K),
            axis=mybir.AxisListType.X, op=alu.add)
        denom = wpool.tile([P, Tn], F32, tag="denom")
        nc.vector.tensor_scalar(denom[:], wnsum[:], 1.0, None, alu.add)
        inv_denom = wpool.tile([P, Tn], F32, tag="inv_denom")
        nc.vector.reciprocal(inv_denom[:], denom[:])
        c1 = wpool.tile([P, Tn], F32, tag="c1")
        nc.vector.tensor_tensor(c1[:], inv_total[:], inv_denom[:], alu.mult)

        if act_ks > 0:
            ones = cpool.tile([P, P], F32, tag="ones")
            nc.vector.memset(ones[:], 1.0)
            ident = cpool.tile([P, P], F32, tag="ident")
            nc.gpsimd.affine_select(
                ident[:], ones[:], [[-1, P]], alu.is_equal, 0.0,
                base=0, channel_multiplier=1)

        dve_ks = K - act_ks
        with tc.tile_pool(name="psum", bufs=4, space="PSUM") as ppool:
            for t in range(Tn):
                G = gpool.tile([P, KP1 * D], F32, tag="G")
                for j in range(KP1):
                    nc.gpsimd.indirect_dma_start(
                        G[:, j * D:(j + 1) * D],
                        None,
                        feats[:, :],
                        IndirectOffsetOnAxis(
                            ap=idx_sb[:, t * KP1 + j:t * KP1 + j + 1], axis=0),
                    )

                wt = w_all[:, t * K:(t + 1) * K]

                acc = apool.tile([P, D], F32, tag="acc")
                nc.vector.tensor_scalar(
                    acc[:], G[:, D:2 * D], wt[:, 0:1], None, alu.mult)
                for k in range(1, dve_ks):
                    nc.vector.scalar_tensor_tensor(
                        acc[:], G[:, (k + 1) * D:(k + 2) * D], wt[:, k:k + 1],
                        acc[:], op0=alu.mult, op1=alu.add)

                if act_ks > 0:
                    pt = ppool.tile([P, D], F32, tag="pt")
                    for i, k in enumerate(range(dve_ks, K)):
                        diag = dpool.tile([P, P], F32, tag="diag")
                        nc.scalar.activation(diag[:], ident[:], act.Copy,
                                             scale=wt[:, k:k + 1])
                        nc.tensor.matmul(
                            pt[:], diag[:], G[:, (k + 1) * D:(k + 2) * D],
                            start=(i == 0), stop=(i == act_ks - 1))
                    nc.vector.tensor_tensor(acc[:], acc[:], pt[:], alu.add)

                # out = G0 * inv_denom + acc * c1
                accs = apool.tile([P, D], F32, tag="accs")
                nc.vector.tensor_scalar(
                    accs[:], acc[:], c1[:, t:t + 1], None, alu.mult)
                ot = opool.tile([P, D], F32, tag="ot")
                nc.vector.scalar_tensor_tensor(
                    ot[:], G[:, 0:D], inv_denom[:, t:t + 1], accs[:],
                    op0=alu.mult, op1=alu.add)
                nc.sync.dma_start(out[t * P:(t + 1) * P, :], ot[:])


_NC = None


def _get_nc():
    global _NC
    if _NC is None:
        nc = bacc.Bacc("TRN2", target_bir_lowering=False, debug=False,
                       enable_asserts=False)
        feats = nc.dram_tensor("features", [N, D], F32,
                               kind="ExternalInput").ap()
        idx = nc.dram_tensor("idx", [P, T * (K + 1)], mybir.dt.int32,
                             kind="ExternalInput").ap()
        times = nc.dram_tensor("times", [P, T * K], F32,
                               kind="ExternalInput").ap()
        out = nc.dram_tensor("out", [B_PAD, D], F32,
                             kind="ExternalOutput").ap()
        with tile.TileContext(nc) as tc:
            _build_kernel(tc, {"out": out},
                          {"features": feats, "idx": idx, "times": times},
                          T, ACT_KS)
        nc.compile()
        _NC = nc
    return _NC


def kernel(nodes, neigh_ids, neigh_times, features):
    global LAST_RESULT
    nodes = np.asarray(nodes).astype(np.int32, copy=False)
    neigh_ids = np.asarray(neigh_ids).astype(np.int32, copy=False)
    neigh_times = np.asarray(neigh_times).astype(np.float32, copy=False)
    features = np.ascontiguousarray(np.asarray(features, dtype=np.float32))

    in_maps = []
    for c in range(N_CORES):
        sl = slice(c * B_CORE, (c + 1) * B_CORE)
        idx_all = np.zeros((B_PAD, K + 1), np.int32)
        idx_all[:B_CORE, 0] = nodes[sl]
        idx_all[:B_CORE, 1:] = neigh_ids[sl]
        times_pad = np.full((B_PAD, K), 200.0, np.float32)
        times_pad[:B_CORE] = neigh_times[sl]
        idx_t = np.ascontiguousarray(
            idx_all.reshape(T, P, K + 1).transpose(1, 0, 2).reshape(P, -1))
        times_t = np.ascontiguousarray(
            times_pad.reshape(T, P, K).transpose(1, 0, 2).reshape(P, -1))
        in_maps.append({"features": features, "idx": idx_t, "times": times_t})

    nc = _get_nc()
    res = run_bass_kernel_spmd(nc, in_maps, core_ids=list(range(N_CORES)))
    LAST_RESULT = res
    out = np.concatenate(
        [res.results[c]["out"][:B_CORE] for c in range(N_CORES)], axis=0)
    return out



# revision 15
# speedup vs baseline: 1.5849x; 1.5849x over previous
"""Trainium2 Bass kernel for nn_MeanAggregator (time-decayed GNN mean aggregation).

Contract: kernel(**inputs) takes the FULL inputs
  nodes [50000] int, neigh_ids [50000,32] int, neigh_times [50000,32] f32,
  features [500000,128] f32
and returns the FULL output [50000,128] f32.

Strategy: data-parallel shard of the batch dim across 8 NeuronCores, feature
table replicated per core. Per 128-row tile, one indirect (gather) DMA per
slot pulls self+neighbor feature rows into SBUF (the HW indirect-DMA ucode
honors exactly one offset per partition per instruction); the time-decay
weights are computed in one batched phase; the weighted sum runs on DVE
(scalar_tensor_tensor chain) with a slice of neighbor slots offloaded to
ScalarE(diag build)+TensorE(PSUM accumulation).

Host-side, neighbors with t > TIME (weight exactly 0) are dropped and nodes
are sorted by valid-neighbor count so each 128-row tile only issues gathers
for the slots it actually needs -- roughly halving the dominant per-gather
SWDGE cost. The kernel is compiled on first call for the observed per-tile
slot counts (cached by that signature).
"""
import sys
import types

import numpy as np

try:
    import antenv.axon_hooks  # noqa: F401
except Exception:
    try:
        import antenv
        _mod = types.ModuleType("antenv.axon_hooks")
        _holder = {"v": None}
        _mod.set_axon_ntff_profile_hook = lambda h: _holder.__setitem__("v", h)
        _mod.get_axon_ntff_profile_hook = lambda: _holder["v"]
        sys.modules["antenv.axon_hooks"] = _mod
        antenv.axon_hooks = _mod
    except Exception:
        pass

import concourse.bacc as bacc
import concourse.mybir as mybir
import concourse.tile as tile
from concourse.bass import IndirectOffsetOnAxis
from concourse.bass_utils import run_bass_kernel_spmd
from concourse.mybir import ActivationFunctionType as act
from concourse.mybir import AluOpType as alu

TIME = 100.0
TAU = 100.0
P = 128
F32 = mybir.dt.float32
F16 = mybir.dt.float16
I32 = mybir.dt.int32

N_CORES = 8
B = 50000
K = 32
N = 500000
D = 128
B_CORE = B // N_CORES          # 6250
T = -(-B_CORE // P)            # 49 tiles
B_PAD = T * P                  # 6272
PADROWS = B_PAD - B_CORE       # 22
SLAB = 4                       # tiles per PSUM group
ACT_KS = 12                    # neighbor slots pre-scaled on ScalarE

LAST_RESULT = None


def _build_kernel(tc, outs, ins, slabs, act_ks=ACT_KS):
    """slabs: [(t0, S, Ks)] with sum(S) == T; Ks = neighbor slots per tile."""
    nc = tc.nc
    feats = ins["features"]
    idx = ins["idx"]
    times = ins["times"]
    out = outs["out"]
    Tn = sum(s for _, s, _ in slabs)
    TMC = sum(s * ks for _, s, ks in slabs)
    ICOLS = sum(s * (ks + 1) for _, s, ks in slabs)
    maxg = max(ks + 1 for _, _, ks in slabs)
    maxs = max(s for _, s, _ in slabs)

    with (
        tc.tile_pool(name="const", bufs=1) as cpool,
        tc.tile_pool(name="scratch", bufs=1) as spool,
        tc.tile_pool(name="gather", bufs=2) as gpool,
        tc.tile_pool(name="wg", bufs=2) as wgpool,
        tc.tile_pool(name="fin", bufs=4) as fpool,
    ):
        idx_sb = cpool.tile([P, ICOLS], I32, tag="idx")
        nc.sync.dma_start(idx_sb[:], idx[:, :])
        times_sb = cpool.tile([P, max(TMC, 1)], F32, tag="times")
        if TMC > 0:
            nc.sync.dma_start(times_sb[:], times[:, :])
        neg1 = cpool.tile([P, 1], F32, tag="neg1")
        nc.vector.memset(neg1[:], -1.0)
        ones32 = cpool.tile([P, P], F32, tag="ones32")
        nc.vector.memset(ones32[:], 1.0)
        ident32 = cpool.tile([P, P], F32, tag="ident32")
        nc.gpsimd.affine_select(
            ident32[:], ones32[:], [[-1, P]], alu.is_equal, 0.0,
            base=0, channel_multiplier=1)
        ident = cpool.tile([P, P], F16, tag="ident")
        nc.vector.tensor_scalar(ident[:], ident32[:], 0.0, None, alu.add)

        # ---- batched weights phase (ragged tile-major layout) ----
        e_all = spool.tile([P, max(TMC, 1)], F32, tag="e")
        nc.scalar.activation(e_all[:], times_sb[:], act.Exp,
                             bias=neg1[:, :], scale=1.0 / TAU)
        mask = spool.tile([P, max(TMC, 1)], F32, tag="mask")
        nc.vector.tensor_scalar(mask[:], times_sb[:], TIME, None, alu.is_le)
        w_all = cpool.tile([P, max(TMC, 1)], F32, tag="w")
        nc.vector.tensor_tensor(w_all[:], e_all[:], mask[:], alu.mult)
        w16 = cpool.tile([P, max(TMC, 1)], F16, tag="w16")
        nc.vector.tensor_scalar(w16[:], w_all[:], 0.0, None, alu.add)

        total = spool.tile([P, Tn], F32, tag="total")
        toff = 0
        for (t0, S, Ks) in slabs:
            if Ks == 0:
                nc.vector.memset(total[:, t0:t0 + S], 0.0)
                continue
            nc.vector.tensor_reduce(
                total[:, t0:t0 + S],
                w_all[:, toff:toff + S * Ks].rearrange(
                    "p (s k) -> p s k", k=Ks),
                axis=mybir.AxisListType.X, op=alu.add)
            toff += S * Ks
        iszero = spool.tile([P, Tn], F32, tag="iszero")
        nc.vector.tensor_scalar(iszero[:], total[:], 0.0, None, alu.is_equal)
        total_adj = spool.tile([P, Tn], F32, tag="tadj")
        nc.vector.tensor_tensor(total_adj[:], total[:], iszero[:], alu.add)
        inv_total = spool.tile([P, Tn], F32, tag="invt")
        nc.vector.reciprocal(inv_total[:], total_adj[:])
        # denom = 1 + sum_k w_k/total is exactly 2 (total>0) or 1 (total==0)
        inv_denom = cpool.tile([P, Tn], F32, tag="invd")
        nc.vector.tensor_scalar(inv_denom[:], iszero[:], 0.5, 0.5,
                                alu.mult, alu.add)
        c1 = cpool.tile([P, Tn], F32, tag="c1")
        nc.vector.tensor_tensor(c1[:], inv_total[:], inv_denom[:], alu.mult)

        with tc.tile_pool(name="psum", bufs=2, space="PSUM") as ppool:
            ioff = 0
            toff = 0
            for (t0, S, Ks) in slabs:
                KQ = Ks + 1
                G = gpool.tile([P, maxs * maxg * D], F16, tag="G")
                for s in range(S):
                    for q in range(KQ):
                        col = ioff + s * KQ + q
                        nc.gpsimd.indirect_dma_start(
                            G[:, (s * KQ + q) * D:(s * KQ + q + 1) * D],
                            None,
                            feats[:, :],
                            IndirectOffsetOnAxis(
                                ap=idx_sb[:, col:col + 1], axis=0),
                        )

                if Ks > 0:
                    a_ks = min(act_ks, Ks)
                    wG = wgpool.tile([P, maxs * (maxg - 1) * D], F16,
                                     tag="wG")
                    for s in range(S):
                        woff = toff + s * Ks
                        for k in range(a_ks):
                            nc.scalar.activation(
                                wG[:, (s * Ks + k) * D:(s * Ks + k + 1) * D],
                                G[:, (s * KQ + 1 + k) * D:
                                  (s * KQ + 2 + k) * D],
                                act.Copy,
                                scale=w_all[:, woff + k:woff + k + 1])
                        if Ks > a_ks:
                            nc.vector.tensor_tensor(
                                wG[:, (s * Ks + a_ks) * D:(s + 1) * Ks * D]
                                .rearrange("p (k d) -> p k d", d=D),
                                G[:, (s * KQ + 1 + a_ks) * D:(s + 1) * KQ * D]
                                .rearrange("p (k d) -> p k d", d=D),
                                w16[:, woff + a_ks:woff + Ks]
                                .to_broadcast((P, Ks - a_ks, D)),
                                alu.mult)

                    psum = ppool.tile([P, SLAB * D], F32, tag="ps")
                    wG_v = wG[:, :S * Ks * D].rearrange(
                        "p (s k d) -> p s k d", k=Ks, d=D)
                    ps_v = psum[:, :S * D].rearrange("p (s d) -> p s d", d=D)
                    for k in range(Ks):
                        nc.tensor.matmul(
                            ps_v, ident[:], wG_v[:, :, k:k + 1, :],
                            start=(k == 0), stop=(k == Ks - 1))

                for s in range(S):
                    t = t0 + s
                    ot = fpool.tile([P, D], F32, tag="ot")
                    if Ks > 0:
                        tmp = fpool.tile([P, D], F32, tag="tmp")
                        nc.vector.tensor_scalar(
                            tmp[:], psum[:, s * D:(s + 1) * D],
                            c1[:, t:t + 1], None, alu.mult)
                        nc.vector.scalar_tensor_tensor(
                            ot[:], G[:, s * KQ * D:(s * KQ + 1) * D],
                            inv_denom[:, t:t + 1], tmp[:],
                            op0=alu.mult, op1=alu.add)
                    else:
                        nc.vector.tensor_scalar(
                            ot[:], G[:, s * KQ * D:(s * KQ + 1) * D],
                            inv_denom[:, t:t + 1], None, alu.mult)
                    nc.sync.dma_start(out[t * P:(t + 1) * P, :], ot[:])
                ioff += S * KQ
                toff += S * Ks


_NC_CACHE = {}


def _get_nc(slabs):
    key = tuple(slabs)
    if key not in _NC_CACHE:
        TMC = sum(s * ks for _, s, ks in slabs)
        ICOLS = sum(s * (ks + 1) for _, s, ks in slabs)
        nc = bacc.Bacc("TRN2", target_bir_lowering=False, debug=False,
                       enable_asserts=False)
        feats = nc.dram_tensor("features", [N, D], F16,
                               kind="ExternalInput").ap()
        idx = nc.dram_tensor("idx", [P, ICOLS], I32,
                             kind="ExternalInput").ap()
        times = nc.dram_tensor("times", [P, max(TMC, 1)], F32,
                               kind="ExternalInput").ap()
        out = nc.dram_tensor("out", [B_PAD, D], F32,
                             kind="ExternalOutput").ap()
        with tile.TileContext(nc) as tc:
            _build_kernel(tc, {"out": out},
                          {"features": feats, "idx": idx, "times": times},
                          slabs)
        nc.compile()
        _NC_CACHE[key] = nc
    return _NC_CACHE[key]


def kernel(nodes, neigh_ids, neigh_times, features):
    global LAST_RESULT
    nodes = np.asarray(nodes).astype(np.int32, copy=False)
    neigh_ids = np.asarray(neigh_ids).astype(np.int32, copy=False)
    neigh_times = np.asarray(neigh_times).astype(np.float32, copy=False)
    features16 = np.ascontiguousarray(
        np.asarray(features, dtype=np.float32)).astype(np.float16)

    # compact valid neighbors (t <= TIME) to the front; dropped slots have
    # weight exactly 0 in the reference, so results are identical
    valid = neigh_times <= TIME
    cnt = valid.sum(1).astype(np.int32)
    ordc = np.argsort(~valid, axis=1, kind='stable')
    nid_c = np.take_along_axis(neigh_ids, ordc, 1)
    nt_c = np.take_along_axis(neigh_times, ordc, 1)
    pos = np.arange(K)[None, :]
    nid_c = np.where(pos < cnt[:, None], nid_c, 0).astype(np.int32)
    nt_c = np.where(pos < cnt[:, None], nt_c, 200.0).astype(np.float32)

    # sort nodes by valid count and deal round-robin so all 8 cores share
    # nearly identical per-tile slot counts (one compiled kernel for all)
    order = np.argsort(cnt, kind='stable')
    core_rows = [order[c::N_CORES] for c in range(N_CORES)]

    nodes_cs, nbr_cs, ts_cs, cnt_cs = [], [], [], []
    for c in range(N_CORES):
        rows = core_rows[c]
        nodes_c = np.zeros(B_PAD, np.int32)
        nodes_c[PADROWS:] = nodes[rows]
        nbr = np.zeros((B_PAD, K), np.int32)
        nbr[PADROWS:] = nid_c[rows]
        ts = np.full((B_PAD, K), 200.0, np.float32)
        ts[PADROWS:] = nt_c[rows]
        cp = np.zeros(B_PAD, np.int32)
        cp[PADROWS:] = cnt[rows]
        nodes_cs.append(nodes_c)
        nbr_cs.append(nbr)
        ts_cs.append(ts)
        cnt_cs.append(cp)

    tile_max = np.stack(
        [cp.reshape(T, P).max(1) for cp in cnt_cs]).max(0)  # [T]
    slabs = []
    t0 = 0
    while t0 < T:
        S = min(SLAB, T - t0)
        Ks = int(tile_max[t0:t0 + S].max())
        slabs.append((t0, S, Ks))
        t0 += S

    in_maps = []
    for c in range(N_CORES):
        idx_parts = []
        tm_parts = []
        for (t0, S, Ks) in slabs:
            r0, r1 = t0 * P, (t0 + S) * P
            blk = np.concatenate(
                [nodes_cs[c][r0:r1, None], nbr_cs[c][r0:r1, :Ks]], axis=1)
            idx_parts.append(
                blk.reshape(S, P, Ks + 1).transpose(1, 0, 2).reshape(P, -1))
            if Ks > 0:
                tb = ts_cs[c][r0:r1, :Ks]
                tm_parts.append(
                    tb.reshape(S, P, Ks).transpose(1, 0, 2).reshape(P, -1))
        idx_t = np.ascontiguousarray(np.concatenate(idx_parts, axis=1))
        times_t = (np.ascontiguousarray(np.concatenate(tm_parts, axis=1))
                   if tm_parts else np.zeros((P, 1), np.float32))
        in_maps.append(
            {"features": features16, "idx": idx_t, "times": times_t})

    nc = _get_nc(slabs)
    res = run_bass_kernel_spmd(nc, in_maps, core_ids=list(range(N_CORES)))
    LAST_RESULT = res
    out = np.empty((B, D), np.float32)
    for c in range(N_CORES):
        out[core_rows[c]] = res.results[c]["out"][PADROWS:]
    return out


# revision 17
# speedup vs baseline: 1.8982x; 1.1977x over previous
"""Trainium2 Bass kernel for nn_MeanAggregator (time-decayed GNN mean aggregation).

Contract: kernel(**inputs) takes the FULL inputs
  nodes [50000] int, neigh_ids [50000,32] int, neigh_times [50000,32] f32,
  features [500000,128] f32
and returns the FULL output [50000,128] f32.

Strategy: data-parallel shard of the batch dim across 8 NeuronCores, feature
table replicated per core. Per 128-row tile, one indirect (gather) DMA per
slot pulls self+neighbor feature rows into SBUF (the HW indirect-DMA ucode
honors exactly one offset per partition per instruction); the time-decay
weights are computed in one batched phase; the weighted sum runs on DVE
(scalar_tensor_tensor chain) with a slice of neighbor slots offloaded to
ScalarE(diag build)+TensorE(PSUM accumulation).

Host-side, neighbors with t > TIME (weight exactly 0) are dropped and nodes
are sorted by valid-neighbor count so each 128-row tile only issues gathers
for the slots it actually needs -- roughly halving the dominant per-gather
SWDGE cost. The kernel is compiled on first call for the observed per-tile
slot counts (cached by that signature).
"""
import sys
import types

import numpy as np

try:
    import antenv.axon_hooks  # noqa: F401
except Exception:
    try:
        import antenv
        _mod = types.ModuleType("antenv.axon_hooks")
        _holder = {"v": None}
        _mod.set_axon_ntff_profile_hook = lambda h: _holder.__setitem__("v", h)
        _mod.get_axon_ntff_profile_hook = lambda: _holder["v"]
        sys.modules["antenv.axon_hooks"] = _mod
        antenv.axon_hooks = _mod
    except Exception:
        pass

import concourse.bacc as bacc
import concourse.mybir as mybir
import concourse.tile as tile
from concourse.bass import IndirectOffsetOnAxis
from concourse.bass_utils import run_bass_kernel_spmd
from concourse.mybir import ActivationFunctionType as act
from concourse.mybir import AluOpType as alu

TIME = 100.0
TAU = 100.0
P = 128
F32 = mybir.dt.float32
F16 = mybir.dt.float16
I32 = mybir.dt.int32

N_CORES = 8
B = 50000
K = 32
N = 500000
D = 128
B_CORE = B // N_CORES          # 6250
T = -(-B_CORE // P)            # 49 tiles
B_PAD = T * P                  # 6272
PADROWS = B_PAD - B_CORE       # 22
SLAB = 2                       # tiles per PSUM group / slot-count granularity
ACT_KS = 12                    # neighbor slots pre-scaled on ScalarE

LAST_RESULT = None


def _build_kernel(tc, outs, ins, slabs, act_ks=ACT_KS):
    """slabs: [(t0, S, Ks)] with sum(S) == T; Ks = neighbor slots per tile."""
    nc = tc.nc
    feats = ins["features"]
    idx = ins["idx"]
    times = ins["times"]
    out = outs["out"]
    Tn = sum(s for _, s, _ in slabs)
    TMC = sum(s * ks for _, s, ks in slabs)
    ICOLS = sum(s * (ks + 1) for _, s, ks in slabs)
    maxg = max(ks + 1 for _, _, ks in slabs)
    maxs = max(s for _, s, _ in slabs)

    with (
        tc.tile_pool(name="const", bufs=1) as cpool,
        tc.tile_pool(name="scratch", bufs=1) as spool,
        tc.tile_pool(name="gather", bufs=3) as gpool,
        tc.tile_pool(name="wg", bufs=2) as wgpool,
        tc.tile_pool(name="fin", bufs=4) as fpool,
    ):
        idx_sb = cpool.tile([P, ICOLS], I32, tag="idx")
        nc.sync.dma_start(idx_sb[:], idx[:, :])
        times_sb = cpool.tile([P, max(TMC, 1)], F32, tag="times")
        if TMC > 0:
            nc.sync.dma_start(times_sb[:], times[:, :])
        neg1 = cpool.tile([P, 1], F32, tag="neg1")
        nc.vector.memset(neg1[:], -1.0)
        ones32 = cpool.tile([P, P], F32, tag="ones32")
        nc.vector.memset(ones32[:], 1.0)
        ident32 = cpool.tile([P, P], F32, tag="ident32")
        nc.gpsimd.affine_select(
            ident32[:], ones32[:], [[-1, P]], alu.is_equal, 0.0,
            base=0, channel_multiplier=1)
        ident = cpool.tile([P, P], F16, tag="ident")
        nc.vector.tensor_scalar(ident[:], ident32[:], 0.0, None, alu.add)

        # ---- batched weights phase (ragged tile-major layout) ----
        e_all = spool.tile([P, max(TMC, 1)], F32, tag="e")
        nc.scalar.activation(e_all[:], times_sb[:], act.Exp,
                             bias=neg1[:, :], scale=1.0 / TAU)
        mask = spool.tile([P, max(TMC, 1)], F32, tag="mask")
        nc.vector.tensor_scalar(mask[:], times_sb[:], TIME, None, alu.is_le)
        w_all = cpool.tile([P, max(TMC, 1)], F32, tag="w")
        nc.vector.tensor_tensor(w_all[:], e_all[:], mask[:], alu.mult)
        w16 = cpool.tile([P, max(TMC, 1)], F16, tag="w16")
        nc.vector.tensor_scalar(w16[:], w_all[:], 0.0, None, alu.add)

        total = spool.tile([P, Tn], F32, tag="total")
        toff = 0
        for (t0, S, Ks) in slabs:
            if Ks == 0:
                nc.vector.memset(total[:, t0:t0 + S], 0.0)
                continue
            nc.vector.tensor_reduce(
                total[:, t0:t0 + S],
                w_all[:, toff:toff + S * Ks].rearrange(
                    "p (s k) -> p s k", k=Ks),
                axis=mybir.AxisListType.X, op=alu.add)
            toff += S * Ks
        iszero = spool.tile([P, Tn], F32, tag="iszero")
        nc.vector.tensor_scalar(iszero[:], total[:], 0.0, None, alu.is_equal)
        total_adj = spool.tile([P, Tn], F32, tag="tadj")
        nc.vector.tensor_tensor(total_adj[:], total[:], iszero[:], alu.add)
        inv_total = spool.tile([P, Tn], F32, tag="invt")
        nc.vector.reciprocal(inv_total[:], total_adj[:])
        # denom = 1 + sum_k w_k/total is exactly 2 (total>0) or 1 (total==0)
        inv_denom = cpool.tile([P, Tn], F32, tag="invd")
        nc.vector.tensor_scalar(inv_denom[:], iszero[:], 0.5, 0.5,
                                alu.mult, alu.add)
        c1 = cpool.tile([P, Tn], F32, tag="c1")
        nc.vector.tensor_tensor(c1[:], inv_total[:], inv_denom[:], alu.mult)

        with tc.tile_pool(name="psum", bufs=2, space="PSUM") as ppool:
            ioff = 0
            toff = 0
            for (t0, S, Ks) in slabs:
                KQ = Ks + 1
                G = gpool.tile([P, maxs * maxg * D], F16, tag="G")
                for s in range(S):
                    for q in range(KQ):
                        col = ioff + s * KQ + q
                        nc.gpsimd.indirect_dma_start(
                            G[:, (s * KQ + q) * D:(s * KQ + q + 1) * D],
                            None,
                            feats[:, :],
                            IndirectOffsetOnAxis(
                                ap=idx_sb[:, col:col + 1], axis=0),
                        )

                if Ks > 0:
                    a_ks = min(act_ks, Ks)
                    wG = wgpool.tile([P, maxs * (maxg - 1) * D], F16,
                                     tag="wG")
                    for s in range(S):
                        woff = toff + s * Ks
                        for k in range(a_ks):
                            nc.scalar.activation(
                                wG[:, (s * Ks + k) * D:(s * Ks + k + 1) * D],
                                G[:, (s * KQ + 1 + k) * D:
                                  (s * KQ + 2 + k) * D],
                                act.Copy,
                                scale=w_all[:, woff + k:woff + k + 1])
                        if Ks > a_ks:
                            nc.vector.tensor_tensor(
                                wG[:, (s * Ks + a_ks) * D:(s + 1) * Ks * D]
                                .rearrange("p (k d) -> p k d", d=D),
                                G[:, (s * KQ + 1 + a_ks) * D:(s + 1) * KQ * D]
                                .rearrange("p (k d) -> p k d", d=D),
                                w16[:, woff + a_ks:woff + Ks]
                                .to_broadcast((P, Ks - a_ks, D)),
                                alu.mult)

                    psum = ppool.tile([P, SLAB * D], F32, tag="ps")
                    wG_v = wG[:, :S * Ks * D].rearrange(
                        "p (s k d) -> p s k d", k=Ks, d=D)
                    ps_v = psum[:, :S * D].rearrange("p (s d) -> p s d", d=D)
                    for k in range(Ks):
                        nc.tensor.matmul(
                            ps_v, ident[:], wG_v[:, :, k:k + 1, :],
                            start=(k == 0), stop=(k == Ks - 1))

                for s in range(S):
                    t = t0 + s
                    ot = fpool.tile([P, D], F32, tag="ot")
                    if Ks > 0:
                        tmp = fpool.tile([P, D], F32, tag="tmp")
                        nc.vector.tensor_scalar(
                            tmp[:], psum[:, s * D:(s + 1) * D],
                            c1[:, t:t + 1], None, alu.mult)
                        nc.vector.scalar_tensor_tensor(
                            ot[:], G[:, s * KQ * D:(s * KQ + 1) * D],
                            inv_denom[:, t:t + 1], tmp[:],
                            op0=alu.mult, op1=alu.add)
                    else:
                        nc.vector.tensor_scalar(
                            ot[:], G[:, s * KQ * D:(s * KQ + 1) * D],
                            inv_denom[:, t:t + 1], None, alu.mult)
                    nc.sync.dma_start(out[t * P:(t + 1) * P, :], ot[:])
                ioff += S * KQ
                toff += S * Ks


_NC_CACHE = {}


def _get_nc(slabs):
    key = tuple(slabs)
    if key not in _NC_CACHE:
        TMC = sum(s * ks for _, s, ks in slabs)
        ICOLS = sum(s * (ks + 1) for _, s, ks in slabs)
        nc = bacc.Bacc("TRN2", target_bir_lowering=False, debug=False,
                       enable_asserts=False)
        feats = nc.dram_tensor("features", [N, D], F16,
                               kind="ExternalInput").ap()
        idx = nc.dram_tensor("idx", [P, ICOLS], I32,
                             kind="ExternalInput").ap()
        times = nc.dram_tensor("times", [P, max(TMC, 1)], F32,
                               kind="ExternalInput").ap()
        out = nc.dram_tensor("out", [B_PAD, D], F32,
                             kind="ExternalOutput").ap()
        with tile.TileContext(nc) as tc:
            _build_kernel(tc, {"out": out},
                          {"features": feats, "idx": idx, "times": times},
                          slabs)
        nc.compile()
        _NC_CACHE[key] = nc
    return _NC_CACHE[key]


def kernel(nodes, neigh_ids, neigh_times, features):
    global LAST_RESULT
    nodes = np.asarray(nodes).astype(np.int32, copy=False)
    neigh_ids = np.asarray(neigh_ids).astype(np.int32, copy=False)
    neigh_times = np.asarray(neigh_times).astype(np.float32, copy=False)
    features16 = np.ascontiguousarray(
        np.asarray(features, dtype=np.float32)).astype(np.float16)

    # compact valid neighbors (t <= TIME) to the front; dropped slots have
    # weight exactly 0 in the reference, so results are identical
    valid = neigh_times <= TIME
    cnt = valid.sum(1).astype(np.int32)
    ordc = np.argsort(~valid, axis=1, kind='stable')
    nid_c = np.take_along_axis(neigh_ids, ordc, 1)
    nt_c = np.take_along_axis(neigh_times, ordc, 1)
    pos = np.arange(K)[None, :]
    nid_c = np.where(pos < cnt[:, None], nid_c, 0).astype(np.int32)
    nt_c = np.where(pos < cnt[:, None], nt_c, 200.0).astype(np.float32)

    # sort nodes by valid count and deal round-robin so all 8 cores share
    # nearly identical per-tile slot counts (one compiled kernel for all)
    order = np.argsort(cnt, kind='stable')
    core_rows = [order[c::N_CORES] for c in range(N_CORES)]

    nodes_cs, nbr_cs, ts_cs, cnt_cs = [], [], [], []
    for c in range(N_CORES):
        rows = core_rows[c]
        nodes_c = np.zeros(B_PAD, np.int32)
        nodes_c[PADROWS:] = nodes[rows]
        nbr = np.zeros((B_PAD, K), np.int32)
        nbr[PADROWS:] = nid_c[rows]
        ts = np.full((B_PAD, K), 200.0, np.float32)
        ts[PADROWS:] = nt_c[rows]
        cp = np.zeros(B_PAD, np.int32)
        cp[PADROWS:] = cnt[rows]
        nodes_cs.append(nodes_c)
        nbr_cs.append(nbr)
        ts_cs.append(ts)
        cnt_cs.append(cp)

    tile_max = np.stack(
        [cp.reshape(T, P).max(1) for cp in cnt_cs]).max(0)  # [T]
    slabs = []
    t0 = 0
    while t0 < T:
        S = min(SLAB, T - t0)
        Ks = int(tile_max[t0:t0 + S].max())
        slabs.append((t0, S, Ks))
        t0 += S

    in_maps = []
    for c in range(N_CORES):
        idx_parts = []
        tm_parts = []
        for (t0, S, Ks) in slabs:
            r0, r1 = t0 * P, (t0 + S) * P
            blk = np.concatenate(
                [nodes_cs[c][r0:r1, None], nbr_cs[c][r0:r1, :Ks]], axis=1)
            idx_parts.append(
                blk.reshape(S, P, Ks + 1).transpose(1, 0, 2).reshape(P, -1))
            if Ks > 0:
                tb = ts_cs[c][r0:r1, :Ks]
                tm_parts.append(
                    tb.reshape(S, P, Ks).transpose(1, 0, 2).reshape(P, -1))
        idx_t = np.ascontiguousarray(np.concatenate(idx_parts, axis=1))
        times_t = (np.ascontiguousarray(np.concatenate(tm_parts, axis=1))
                   if tm_parts else np.zeros((P, 1), np.float32))
        in_maps.append(
            {"features": features16, "idx": idx_t, "times": times_t})

    nc = _get_nc(slabs)
    res = run_bass_kernel_spmd(nc, in_maps, core_ids=list(range(N_CORES)))
    LAST_RESULT = res
    out = np.empty((B, D), np.float32)
    for c in range(N_CORES):
        out[core_rows[c]] = res.results[c]["out"][PADROWS:]
    return out


# revision 19
# speedup vs baseline: 1.9036x; 1.0029x over previous
"""Trainium2 Bass kernel for nn_MeanAggregator (time-decayed GNN mean aggregation).

Contract: kernel(**inputs) takes the FULL inputs
  nodes [50000] int, neigh_ids [50000,32] int, neigh_times [50000,32] f32,
  features [500000,128] f32
and returns the FULL output [50000,128] f32.

Strategy: data-parallel shard of the batch dim across 8 NeuronCores, feature
table replicated per core. Per 128-row tile, one indirect (gather) DMA per
slot pulls self+neighbor feature rows into SBUF (the HW indirect-DMA ucode
honors exactly one offset per partition per instruction); the time-decay
weights are computed in one batched phase; the weighted sum runs on DVE
(scalar_tensor_tensor chain) with a slice of neighbor slots offloaded to
ScalarE(diag build)+TensorE(PSUM accumulation).

Host-side, neighbors with t > TIME (weight exactly 0) are dropped and nodes
are sorted by valid-neighbor count so each 128-row tile only issues gathers
for the slots it actually needs -- roughly halving the dominant per-gather
SWDGE cost. The kernel is compiled on first call for the observed per-tile
slot counts (cached by that signature).
"""
import sys
import types

import numpy as np

try:
    import antenv.axon_hooks  # noqa: F401
except Exception:
    try:
        import antenv
        _mod = types.ModuleType("antenv.axon_hooks")
        _holder = {"v": None}
        _mod.set_axon_ntff_profile_hook = lambda h: _holder.__setitem__("v", h)
        _mod.get_axon_ntff_profile_hook = lambda: _holder["v"]
        sys.modules["antenv.axon_hooks"] = _mod
        antenv.axon_hooks = _mod
    except Exception:
        pass

import concourse.bacc as bacc
import concourse.mybir as mybir
import concourse.tile as tile
from concourse.bass import IndirectOffsetOnAxis
from concourse.bass_utils import run_bass_kernel_spmd
from concourse.mybir import ActivationFunctionType as act
from concourse.mybir import AluOpType as alu

TIME = 100.0
TAU = 100.0
P = 128
F32 = mybir.dt.float32
F16 = mybir.dt.float16
I32 = mybir.dt.int32

N_CORES = 8
B = 50000
K = 32
N = 500000
D = 128
B_CORE = B // N_CORES          # 6250
T = -(-B_CORE // P)            # 49 tiles
B_PAD = T * P                  # 6272
PADROWS = B_PAD - B_CORE       # 22
SLAB = 1                       # tiles per PSUM group / slot-count granularity
ACT_KS = 12                    # neighbor slots pre-scaled on ScalarE

LAST_RESULT = None


def _build_kernel(tc, outs, ins, slabs, act_ks=ACT_KS):
    """slabs: [(t0, S, Ks)] with sum(S) == T; Ks = neighbor slots per tile."""
    nc = tc.nc
    feats = ins["features"]
    idx = ins["idx"]
    times = ins["times"]
    out = outs["out"]
    Tn = sum(s for _, s, _ in slabs)
    TMC = sum(s * ks for _, s, ks in slabs)
    ICOLS = sum(s * (ks + 1) for _, s, ks in slabs)
    maxg = max(ks + 1 for _, _, ks in slabs)
    maxs = max(s for _, s, _ in slabs)

    with (
        tc.tile_pool(name="const", bufs=1) as cpool,
        tc.tile_pool(name="scratch", bufs=1) as spool,
        tc.tile_pool(name="gather", bufs=4) as gpool,
        tc.tile_pool(name="wg", bufs=2) as wgpool,
        tc.tile_pool(name="fin", bufs=4) as fpool,
    ):
        idx_sb = cpool.tile([P, ICOLS], I32, tag="idx")
        nc.sync.dma_start(idx_sb[:], idx[:, :])
        times_sb = cpool.tile([P, max(TMC, 1)], F32, tag="times")
        if TMC > 0:
            nc.sync.dma_start(times_sb[:], times[:, :])
        neg1 = cpool.tile([P, 1], F32, tag="neg1")
        nc.vector.memset(neg1[:], -1.0)
        ones32 = cpool.tile([P, P], F32, tag="ones32")
        nc.vector.memset(ones32[:], 1.0)
        ident32 = cpool.tile([P, P], F32, tag="ident32")
        nc.gpsimd.affine_select(
            ident32[:], ones32[:], [[-1, P]], alu.is_equal, 0.0,
            base=0, channel_multiplier=1)
        ident = cpool.tile([P, P], F16, tag="ident")
        nc.vector.tensor_scalar(ident[:], ident32[:], 0.0, None, alu.add)

        # ---- batched weights phase (ragged tile-major layout) ----
        e_all = spool.tile([P, max(TMC, 1)], F32, tag="e")
        nc.scalar.activation(e_all[:], times_sb[:], act.Exp,
                             bias=neg1[:, :], scale=1.0 / TAU)
        mask = spool.tile([P, max(TMC, 1)], F32, tag="mask")
        nc.vector.tensor_scalar(mask[:], times_sb[:], TIME, None, alu.is_le)
        w_all = cpool.tile([P, max(TMC, 1)], F32, tag="w")
        nc.vector.tensor_tensor(w_all[:], e_all[:], mask[:], alu.mult)
        w16 = cpool.tile([P, max(TMC, 1)], F16, tag="w16")
        nc.vector.tensor_scalar(w16[:], w_all[:], 0.0, None, alu.add)

        total = spool.tile([P, Tn], F32, tag="total")
        toff = 0
        for (t0, S, Ks) in slabs:
            if Ks == 0:
                nc.vector.memset(total[:, t0:t0 + S], 0.0)
                continue
            nc.vector.tensor_reduce(
                total[:, t0:t0 + S],
                w_all[:, toff:toff + S * Ks].rearrange(
                    "p (s k) -> p s k", k=Ks),
                axis=mybir.AxisListType.X, op=alu.add)
            toff += S * Ks
        iszero = spool.tile([P, Tn], F32, tag="iszero")
        nc.vector.tensor_scalar(iszero[:], total[:], 0.0, None, alu.is_equal)
        total_adj = spool.tile([P, Tn], F32, tag="tadj")
        nc.vector.tensor_tensor(total_adj[:], total[:], iszero[:], alu.add)
        inv_total = spool.tile([P, Tn], F32, tag="invt")
        nc.vector.reciprocal(inv_total[:], total_adj[:])
        # denom = 1 + sum_k w_k/total is exactly 2 (total>0) or 1 (total==0)
        inv_denom = cpool.tile([P, Tn], F32, tag="invd")
        nc.vector.tensor_scalar(inv_denom[:], iszero[:], 0.5, 0.5,
                                alu.mult, alu.add)
        c1 = cpool.tile([P, Tn], F32, tag="c1")
        nc.vector.tensor_tensor(c1[:], inv_total[:], inv_denom[:], alu.mult)

        with tc.tile_pool(name="psum", bufs=2, space="PSUM") as ppool:
            ioff = 0
            toff = 0
            for (t0, S, Ks) in slabs:
                KQ = Ks + 1
                G = gpool.tile([P, maxs * maxg * D], F16, tag="G")
                for s in range(S):
                    for q in range(KQ):
                        col = ioff + s * KQ + q
                        nc.gpsimd.indirect_dma_start(
                            G[:, (s * KQ + q) * D:(s * KQ + q + 1) * D],
                            None,
                            feats[:, :],
                            IndirectOffsetOnAxis(
                                ap=idx_sb[:, col:col + 1], axis=0),
                        )

                if Ks > 0:
                    a_ks = min(act_ks, Ks)
                    wG = wgpool.tile([P, maxs * (maxg - 1) * D], F16,
                                     tag="wG")
                    for s in range(S):
                        woff = toff + s * Ks
                        for k in range(a_ks):
                            nc.scalar.activation(
                                wG[:, (s * Ks + k) * D:(s * Ks + k + 1) * D],
                                G[:, (s * KQ + 1 + k) * D:
                                  (s * KQ + 2 + k) * D],
                                act.Copy,
                                scale=w_all[:, woff + k:woff + k + 1])
                        if Ks > a_ks:
                            nc.vector.tensor_tensor(
                                wG[:, (s * Ks + a_ks) * D:(s + 1) * Ks * D]
                                .rearrange("p (k d) -> p k d", d=D),
                                G[:, (s * KQ + 1 + a_ks) * D:(s + 1) * KQ * D]
                                .rearrange("p (k d) -> p k d", d=D),
                                w16[:, woff + a_ks:woff + Ks]
                                .to_broadcast((P, Ks - a_ks, D)),
                                alu.mult)

                    psum = ppool.tile([P, SLAB * D], F32, tag="ps")
                    wG_v = wG[:, :S * Ks * D].rearrange(
                        "p (s k d) -> p s k d", k=Ks, d=D)
                    ps_v = psum[:, :S * D].rearrange("p (s d) -> p s d", d=D)
                    for k in range(Ks):
                        nc.tensor.matmul(
                            ps_v, ident[:], wG_v[:, :, k:k + 1, :],
                            start=(k == 0), stop=(k == Ks - 1))

                for s in range(S):
                    t = t0 + s
                    ot = fpool.tile([P, D], F32, tag="ot")
                    if Ks > 0:
                        tmp = fpool.tile([P, D], F32, tag="tmp")
                        nc.vector.tensor_scalar(
                            tmp[:], psum[:, s * D:(s + 1) * D],
                            c1[:, t:t + 1], None, alu.mult)
                        nc.vector.scalar_tensor_tensor(
                            ot[:], G[:, s * KQ * D:(s * KQ + 1) * D],
                            inv_denom[:, t:t + 1], tmp[:],
                            op0=alu.mult, op1=alu.add)
                    else:
                        nc.vector.tensor_scalar(
                            ot[:], G[:, s * KQ * D:(s * KQ + 1) * D],
                            inv_denom[:, t:t + 1], None, alu.mult)
                    nc.sync.dma_start(out[t * P:(t + 1) * P, :], ot[:])
                ioff += S * KQ
                toff += S * Ks


_NC_CACHE = {}


def _get_nc(slabs):
    key = tuple(slabs)
    if key not in _NC_CACHE:
        TMC = sum(s * ks for _, s, ks in slabs)
        ICOLS = sum(s * (ks + 1) for _, s, ks in slabs)
        nc = bacc.Bacc("TRN2", target_bir_lowering=False, debug=False,
                       enable_asserts=False)
        feats = nc.dram_tensor("features", [N, D], F16,
                               kind="ExternalInput").ap()
        idx = nc.dram_tensor("idx", [P, ICOLS], I32,
                             kind="ExternalInput").ap()
        times = nc.dram_tensor("times", [P, max(TMC, 1)], F32,
                               kind="ExternalInput").ap()
        out = nc.dram_tensor("out", [B_PAD, D], F32,
                             kind="ExternalOutput").ap()
        with tile.TileContext(nc) as tc:
            _build_kernel(tc, {"out": out},
                          {"features": feats, "idx": idx, "times": times},
                          slabs)
        nc.compile()
        _NC_CACHE[key] = nc
    return _NC_CACHE[key]


def kernel(nodes, neigh_ids, neigh_times, features):
    global LAST_RESULT
    nodes = np.asarray(nodes).astype(np.int32, copy=False)
    neigh_ids = np.asarray(neigh_ids).astype(np.int32, copy=False)
    neigh_times = np.asarray(neigh_times).astype(np.float32, copy=False)
    features16 = np.ascontiguousarray(
        np.asarray(features, dtype=np.float32)).astype(np.float16)

    # compact valid neighbors (t <= TIME) to the front; dropped slots have
    # weight exactly 0 in the reference, so results are identical
    valid = neigh_times <= TIME
    cnt = valid.sum(1).astype(np.int32)
    ordc = np.argsort(~valid, axis=1, kind='stable')
    nid_c = np.take_along_axis(neigh_ids, ordc, 1)
    nt_c = np.take_along_axis(neigh_times, ordc, 1)
    pos = np.arange(K)[None, :]
    nid_c = np.where(pos < cnt[:, None], nid_c, 0).astype(np.int32)
    nt_c = np.where(pos < cnt[:, None], nt_c, 200.0).astype(np.float32)

    # sort nodes by valid count and deal round-robin so all 8 cores share
    # nearly identical per-tile slot counts (one compiled kernel for all)
    order = np.argsort(cnt, kind='stable')
    core_rows = [order[c::N_CORES] for c in range(N_CORES)]

    nodes_cs, nbr_cs, ts_cs, cnt_cs = [], [], [], []
    for c in range(N_CORES):
        rows = core_rows[c]
        nodes_c = np.zeros(B_PAD, np.int32)
        nodes_c[PADROWS:] = nodes[rows]
        nbr = np.zeros((B_PAD, K), np.int32)
        nbr[PADROWS:] = nid_c[rows]
        ts = np.full((B_PAD, K), 200.0, np.float32)
        ts[PADROWS:] = nt_c[rows]
        cp = np.zeros(B_PAD, np.int32)
        cp[PADROWS:] = cnt[rows]
        nodes_cs.append(nodes_c)
        nbr_cs.append(nbr)
        ts_cs.append(ts)
        cnt_cs.append(cp)

    tile_max = np.stack(
        [cp.reshape(T, P).max(1) for cp in cnt_cs]).max(0)  # [T]
    slabs = []
    t0 = 0
    while t0 < T:
        S = min(SLAB, T - t0)
        Ks = int(tile_max[t0:t0 + S].max())
        slabs.append((t0, S, Ks))
        t0 += S

    in_maps = []
    for c in range(N_CORES):
        idx_parts = []
        tm_parts = []
        for (t0, S, Ks) in slabs:
            r0, r1 = t0 * P, (t0 + S) * P
            blk = np.concatenate(
                [nodes_cs[c][r0:r1, None], nbr_cs[c][r0:r1, :Ks]], axis=1)
            idx_parts.append(
                blk.reshape(S, P, Ks + 1).transpose(1, 0, 2).reshape(P, -1))
            if Ks > 0:
                tb = ts_cs[c][r0:r1, :Ks]
                tm_parts.append(
                    tb.reshape(S, P, Ks).transpose(1, 0, 2).reshape(P, -1))
        idx_t = np.ascontiguousarray(np.concatenate(idx_parts, axis=1))
        times_t = (np.ascontiguousarray(np.concatenate(tm_parts, axis=1))
                   if tm_parts else np.zeros((P, 1), np.float32))
        in_maps.append(
            {"features": features16, "idx": idx_t, "times": times_t})

    nc = _get_nc(slabs)
    res = run_bass_kernel_spmd(nc, in_maps, core_ids=list(range(N_CORES)))
    LAST_RESULT = res
    out = np.empty((B, D), np.float32)
    for c in range(N_CORES):
        out[core_rows[c]] = res.results[c]["out"][PADROWS:]
    return out


# revision 25
# speedup vs baseline: 1.9190x; 1.0081x over previous
"""Trainium2 Bass kernel for nn_MeanAggregator (time-decayed GNN mean aggregation).

Contract: kernel(**inputs) takes the FULL inputs
  nodes [50000] int, neigh_ids [50000,32] int, neigh_times [50000,32] f32,
  features [500000,128] f32
and returns the FULL output [50000,128] f32.

Strategy: data-parallel shard of the batch dim across 8 NeuronCores, feature
table replicated per core. Per 128-row tile, one indirect (gather) DMA per
slot pulls self+neighbor feature rows into SBUF (the HW indirect-DMA ucode
honors exactly one offset per partition per instruction); the time-decay
weights are computed in one batched phase; the weighted sum runs on DVE
(scalar_tensor_tensor chain) with a slice of neighbor slots offloaded to
ScalarE(diag build)+TensorE(PSUM accumulation).

Host-side, neighbors with t > TIME (weight exactly 0) are dropped and nodes
are sorted by valid-neighbor count so each 128-row tile only issues gathers
for the slots it actually needs -- roughly halving the dominant per-gather
SWDGE cost. The kernel is compiled on first call for the observed per-tile
slot counts (cached by that signature).
"""
import sys
import types

import numpy as np

try:
    import antenv.axon_hooks  # noqa: F401
except Exception:
    try:
        import antenv
        _mod = types.ModuleType("antenv.axon_hooks")
        _holder = {"v": None}
        _mod.set_axon_ntff_profile_hook = lambda h: _holder.__setitem__("v", h)
        _mod.get_axon_ntff_profile_hook = lambda: _holder["v"]
        sys.modules["antenv.axon_hooks"] = _mod
        antenv.axon_hooks = _mod
    except Exception:
        pass

import concourse.bacc as bacc
import concourse.mybir as mybir
import concourse.tile as tile
from concourse.bass import IndirectOffsetOnAxis
from concourse.bass_utils import run_bass_kernel_spmd
from concourse.mybir import ActivationFunctionType as act
from concourse.mybir import AluOpType as alu

TIME = 100.0
TAU = 100.0
P = 128
F32 = mybir.dt.float32
F16 = mybir.dt.float16
I32 = mybir.dt.int32

N_CORES = 8
B = 50000
K = 32
N = 500000
D = 128
B_CORE = B // N_CORES          # 6250
T = -(-B_CORE // P)            # 49 tiles
B_PAD = T * P                  # 6272
PADROWS = B_PAD - B_CORE       # 22
SLAB = 1                       # tiles per PSUM group / slot-count granularity
ACT_KS = 12                    # neighbor slots pre-scaled on ScalarE

LAST_RESULT = None


def _build_kernel(tc, outs, ins, slabs, act_ks=ACT_KS):
    """slabs: [(t0, S, Ks)] with sum(S) == T; Ks = neighbor slots per tile."""
    nc = tc.nc
    feats = ins["features"]
    idx = ins["idx"]
    times = ins["times"]
    out = outs["out"]
    Tn = sum(s for _, s, _ in slabs)
    TMC = sum(s * ks for _, s, ks in slabs)
    ICOLS = sum(s * (ks + 1) for _, s, ks in slabs)
    maxg = max(ks + 1 for _, _, ks in slabs)
    maxs = max(s for _, s, _ in slabs)

    with (
        tc.tile_pool(name="const", bufs=1) as cpool,
        tc.tile_pool(name="scratch", bufs=1) as spool,
        tc.tile_pool(name="gather", bufs=8) as gpool,
        tc.tile_pool(name="wg", bufs=4) as wgpool,
        tc.tile_pool(name="fin", bufs=8) as fpool,
    ):
        idx_sb = cpool.tile([P, ICOLS], I32, tag="idx")
        nc.sync.dma_start(idx_sb[:], idx[:, :])
        times_sb = cpool.tile([P, max(TMC, 1)], F32, tag="times")
        if TMC > 0:
            nc.sync.dma_start(times_sb[:], times[:, :])
        neg1 = cpool.tile([P, 1], F32, tag="neg1")
        nc.vector.memset(neg1[:], -1.0)
        ident = cpool.tile([P, P], F16, tag="ident")
        nc.sync.dma_start(ident[:], ins["ident"][:, :])

        # ---- batched weights phase (ragged tile-major layout) ----
        e_all = spool.tile([P, max(TMC, 1)], F32, tag="e")
        nc.scalar.activation(e_all[:], times_sb[:], act.Exp,
                             bias=neg1[:, :], scale=1.0 / TAU)
        mask = spool.tile([P, max(TMC, 1)], F32, tag="mask")
        nc.vector.tensor_scalar(mask[:], times_sb[:], TIME, None, alu.is_le)
        w_all = cpool.tile([P, max(TMC, 1)], F32, tag="w")
        nc.vector.tensor_tensor(w_all[:], e_all[:], mask[:], alu.mult)
        w16 = cpool.tile([P, max(TMC, 1)], F16, tag="w16")
        nc.vector.tensor_scalar(w16[:], w_all[:], 0.0, None, alu.add)

        total = spool.tile([P, Tn], F32, tag="total")
        toff = 0
        for (t0, S, Ks) in slabs:
            if Ks == 0:
                nc.vector.memset(total[:, t0:t0 + S], 0.0)
                continue
            nc.vector.tensor_reduce(
                total[:, t0:t0 + S],
                w_all[:, toff:toff + S * Ks].rearrange(
                    "p (s k) -> p s k", k=Ks),
                axis=mybir.AxisListType.X, op=alu.add)
            toff += S * Ks
        iszero = spool.tile([P, Tn], F32, tag="iszero")
        nc.vector.tensor_scalar(iszero[:], total[:], 0.0, None, alu.is_equal)
        total_adj = spool.tile([P, Tn], F32, tag="tadj")
        nc.vector.tensor_tensor(total_adj[:], total[:], iszero[:], alu.add)
        inv_total = spool.tile([P, Tn], F32, tag="invt")
        nc.vector.reciprocal(inv_total[:], total_adj[:])
        # denom = 1 + sum_k w_k/total is exactly 2 (total>0) or 1 (total==0)
        inv_denom = cpool.tile([P, Tn], F32, tag="invd")
        nc.vector.tensor_scalar(inv_denom[:], iszero[:], 0.5, 0.5,
                                alu.mult, alu.add)
        c1 = cpool.tile([P, Tn], F32, tag="c1")
        nc.vector.tensor_tensor(c1[:], inv_total[:], inv_denom[:], alu.mult)

        with tc.tile_pool(name="psum", bufs=4, space="PSUM") as ppool:
            ioff = 0
            toff = 0
            for (t0, S, Ks) in slabs:
                KQ = Ks + 1
                G = gpool.tile([P, maxs * maxg * D], F16, tag="G")
                for s in range(S):
                    for q in range(KQ):
                        col = ioff + s * KQ + q
                        nc.gpsimd.indirect_dma_start(
                            G[:, (s * KQ + q) * D:(s * KQ + q + 1) * D],
                            None,
                            feats[:, :],
                            IndirectOffsetOnAxis(
                                ap=idx_sb[:, col:col + 1], axis=0),
                        )

                if Ks > 0:
                    a_ks = min(act_ks, Ks)
                    wG = wgpool.tile([P, maxs * (maxg - 1) * D], F16,
                                     tag="wG")
                    for s in range(S):
                        woff = toff + s * Ks
                        for k in range(a_ks):
                            nc.scalar.activation(
                                wG[:, (s * Ks + k) * D:(s * Ks + k + 1) * D],
                                G[:, (s * KQ + 1 + k) * D:
                                  (s * KQ + 2 + k) * D],
                                act.Copy,
                                scale=w_all[:, woff + k:woff + k + 1])
                        if Ks > a_ks:
                            nc.vector.tensor_tensor(
                                wG[:, (s * Ks + a_ks) * D:(s + 1) * Ks * D]
                                .rearrange("p (k d) -> p k d", d=D),
                                G[:, (s * KQ + 1 + a_ks) * D:(s + 1) * KQ * D]
                                .rearrange("p (k d) -> p k d", d=D),
                                w16[:, woff + a_ks:woff + Ks]
                                .to_broadcast((P, Ks - a_ks, D)),
                                alu.mult)

                    psum = ppool.tile([P, SLAB * D], F32, tag="ps")
                    wG_v = wG[:, :S * Ks * D].rearrange(
                        "p (s k d) -> p s k d", k=Ks, d=D)
                    ps_v = psum[:, :S * D].rearrange("p (s d) -> p s d", d=D)
                    for k in range(Ks):
                        nc.tensor.matmul(
                            ps_v, ident[:], wG_v[:, :, k:k + 1, :],
                            start=(k == 0), stop=(k == Ks - 1))

                for s in range(S):
                    t = t0 + s
                    ot = fpool.tile([P, D], F32, tag="ot")
                    if Ks > 0:
                        tmp = fpool.tile([P, D], F32, tag="tmp")
                        nc.vector.tensor_scalar(
                            tmp[:], psum[:, s * D:(s + 1) * D],
                            c1[:, t:t + 1], None, alu.mult)
                        nc.vector.scalar_tensor_tensor(
                            ot[:], G[:, s * KQ * D:(s * KQ + 1) * D],
                            inv_denom[:, t:t + 1], tmp[:],
                            op0=alu.mult, op1=alu.add)
                    else:
                        nc.vector.tensor_scalar(
                            ot[:], G[:, s * KQ * D:(s * KQ + 1) * D],
                            inv_denom[:, t:t + 1], None, alu.mult)
                    nc.sync.dma_start(out[t * P:(t + 1) * P, :], ot[:])
                ioff += S * KQ
                toff += S * Ks


_NC_CACHE = {}


def _get_nc(slabs):
    key = tuple(slabs)
    if key not in _NC_CACHE:
        TMC = sum(s * ks for _, s, ks in slabs)
        ICOLS = sum(s * (ks + 1) for _, s, ks in slabs)
        nc = bacc.Bacc("TRN2", target_bir_lowering=False, debug=False,
                       enable_asserts=False)
        feats = nc.dram_tensor("features", [N, D], F16,
                               kind="ExternalInput").ap()
        idx = nc.dram_tensor("idx", [P, ICOLS], I32,
                             kind="ExternalInput").ap()
        identd = nc.dram_tensor("ident", [P, P], F16,
                                kind="ExternalInput").ap()
        times = nc.dram_tensor("times", [P, max(TMC, 1)], F32,
                               kind="ExternalInput").ap()
        out = nc.dram_tensor("out", [B_PAD, D], F32,
                             kind="ExternalOutput").ap()
        with tile.TileContext(nc) as tc:
            _build_kernel(tc, {"out": out},
                          {"features": feats, "idx": idx, "times": times,
                           "ident": identd},
                          slabs)
        nc.compile()
        _NC_CACHE[key] = nc
    return _NC_CACHE[key]


def kernel(nodes, neigh_ids, neigh_times, features):
    global LAST_RESULT
    nodes = np.asarray(nodes).astype(np.int32, copy=False)
    neigh_ids = np.asarray(neigh_ids).astype(np.int32, copy=False)
    neigh_times = np.asarray(neigh_times).astype(np.float32, copy=False)
    features16 = np.ascontiguousarray(
        np.asarray(features, dtype=np.float32)).astype(np.float16)

    # compact valid neighbors (t <= TIME) to the front; dropped slots have
    # weight exactly 0 in the reference, so results are identical
    valid = neigh_times <= TIME
    cnt = valid.sum(1).astype(np.int32)
    ordc = np.argsort(~valid, axis=1, kind='stable')
    nid_c = np.take_along_axis(neigh_ids, ordc, 1)
    nt_c = np.take_along_axis(neigh_times, ordc, 1)
    pos = np.arange(K)[None, :]
    nid_c = np.where(pos < cnt[:, None], nid_c, 0).astype(np.int32)
    nt_c = np.where(pos < cnt[:, None], nt_c, 200.0).astype(np.float32)

    # sort nodes by valid count and deal round-robin so all 8 cores share
    # nearly identical per-tile slot counts (one compiled kernel for all)
    order = np.argsort(cnt, kind='stable')
    core_rows = [order[c::N_CORES] for c in range(N_CORES)]

    nodes_cs, nbr_cs, ts_cs, cnt_cs = [], [], [], []
    for c in range(N_CORES):
        rows = core_rows[c]
        nodes_c = np.zeros(B_PAD, np.int32)
        nodes_c[PADROWS:] = nodes[rows]
        nbr = np.zeros((B_PAD, K), np.int32)
        nbr[PADROWS:] = nid_c[rows]
        ts = np.full((B_PAD, K), 200.0, np.float32)
        ts[PADROWS:] = nt_c[rows]
        cp = np.zeros(B_PAD, np.int32)
        cp[PADROWS:] = cnt[rows]
        nodes_cs.append(nodes_c)
        nbr_cs.append(nbr)
        ts_cs.append(ts)
        cnt_cs.append(cp)

    tile_max = np.stack(
        [cp.reshape(T, P).max(1) for cp in cnt_cs]).max(0)  # [T]
    slabs = []
    t0 = 0
    while t0 < T:
        S = min(SLAB, T - t0)
        Ks = int(tile_max[t0:t0 + S].max())
        slabs.append((t0, S, Ks))
        t0 += S

    in_maps = []
    for c in range(N_CORES):
        idx_parts = []
        tm_parts = []
        for (t0, S, Ks) in slabs:
            r0, r1 = t0 * P, (t0 + S) * P
            blk = np.concatenate(
                [nodes_cs[c][r0:r1, None], nbr_cs[c][r0:r1, :Ks]], axis=1)
            idx_parts.append(
                blk.reshape(S, P, Ks + 1).transpose(1, 0, 2).reshape(P, -1))
            if Ks > 0:
                tb = ts_cs[c][r0:r1, :Ks]
                tm_parts.append(
                    tb.reshape(S, P, Ks).transpose(1, 0, 2).reshape(P, -1))
        idx_t = np.ascontiguousarray(np.concatenate(idx_parts, axis=1))
        times_t = (np.ascontiguousarray(np.concatenate(tm_parts, axis=1))
                   if tm_parts else np.zeros((P, 1), np.float32))
        in_maps.append(
            {"features": features16, "idx": idx_t, "times": times_t,
             "ident": np.eye(P, dtype=np.float16)})

    nc = _get_nc(slabs)
    res = run_bass_kernel_spmd(nc, in_maps, core_ids=list(range(N_CORES)))
    LAST_RESULT = res
    out = np.empty((B, D), np.float32)
    for c in range(N_CORES):
        out[core_rows[c]] = res.results[c]["out"][PADROWS:]
    return out


# revision 26
# speedup vs baseline: 1.9230x; 1.0021x over previous
"""Trainium2 Bass kernel for nn_MeanAggregator (time-decayed GNN mean aggregation).

Contract: kernel(**inputs) takes the FULL inputs
  nodes [50000] int, neigh_ids [50000,32] int, neigh_times [50000,32] f32,
  features [500000,128] f32
and returns the FULL output [50000,128] f32.

Strategy: data-parallel shard of the batch dim across 8 NeuronCores, feature
table replicated per core. Per 128-row tile, one indirect (gather) DMA per
slot pulls self+neighbor feature rows into SBUF (the HW indirect-DMA ucode
honors exactly one offset per partition per instruction); the time-decay
weights are computed in one batched phase; the weighted sum runs on DVE
(scalar_tensor_tensor chain) with a slice of neighbor slots offloaded to
ScalarE(diag build)+TensorE(PSUM accumulation).

Host-side, neighbors with t > TIME (weight exactly 0) are dropped and nodes
are sorted by valid-neighbor count so each 128-row tile only issues gathers
for the slots it actually needs -- roughly halving the dominant per-gather
SWDGE cost. The kernel is compiled on first call for the observed per-tile
slot counts (cached by that signature).
"""
import sys
import types

import numpy as np

try:
    import antenv.axon_hooks  # noqa: F401
except Exception:
    try:
        import antenv
        _mod = types.ModuleType("antenv.axon_hooks")
        _holder = {"v": None}
        _mod.set_axon_ntff_profile_hook = lambda h: _holder.__setitem__("v", h)
        _mod.get_axon_ntff_profile_hook = lambda: _holder["v"]
        sys.modules["antenv.axon_hooks"] = _mod
        antenv.axon_hooks = _mod
    except Exception:
        pass

import concourse.bacc as bacc
import concourse.mybir as mybir
import concourse.tile as tile
from concourse.bass import IndirectOffsetOnAxis
from concourse.bass_utils import run_bass_kernel_spmd
from concourse.mybir import ActivationFunctionType as act
from concourse.mybir import AluOpType as alu

TIME = 100.0
TAU = 100.0
P = 128
F32 = mybir.dt.float32
F16 = mybir.dt.float16
I32 = mybir.dt.int32

N_CORES = 8
B = 50000
K = 32
N = 500000
D = 128
B_CORE = B // N_CORES          # 6250
T = -(-B_CORE // P)            # 49 tiles
B_PAD = T * P                  # 6272
PADROWS = B_PAD - B_CORE       # 22
SLAB = 1                       # tiles per PSUM group / slot-count granularity
ACT_KS = 12                    # neighbor slots pre-scaled on ScalarE

LAST_RESULT = None


def _build_kernel(tc, outs, ins, slabs, act_ks=ACT_KS):
    """slabs: [(t0, S, Ks)] with sum(S) == T; Ks = neighbor slots per tile."""
    nc = tc.nc
    feats = ins["features"]
    idx = ins["idx"]
    times = ins["times"]
    out = outs["out"]
    Tn = sum(s for _, s, _ in slabs)
    TMC = sum(s * ks for _, s, ks in slabs)
    ICOLS = sum(s * (ks + 1) for _, s, ks in slabs)
    maxg = max(ks + 1 for _, _, ks in slabs)
    maxs = max(s for _, s, _ in slabs)

    with (
        tc.tile_pool(name="const", bufs=1) as cpool,
        tc.tile_pool(name="scratch", bufs=1) as spool,
        tc.tile_pool(name="gather", bufs=8) as gpool,
        tc.tile_pool(name="wg", bufs=4) as wgpool,
        tc.tile_pool(name="fin", bufs=8) as fpool,
    ):
        idx_sb = cpool.tile([P, ICOLS], I32, tag="idx")
        nc.sync.dma_start(idx_sb[:], idx[:, :])
        times_sb = cpool.tile([P, max(TMC, 1)], F32, tag="times")
        if TMC > 0:
            nc.sync.dma_start(times_sb[:], times[:, :])
        neg1 = cpool.tile([P, 1], F32, tag="neg1")
        nc.vector.memset(neg1[:], -1.0)
        ident = cpool.tile([P, P], F16, tag="ident")
        nc.sync.dma_start(ident[:], ins["ident"][:, :])

        # ---- batched weights phase (ragged tile-major layout) ----
        e_all = spool.tile([P, max(TMC, 1)], F32, tag="e")
        nc.scalar.activation(e_all[:], times_sb[:], act.Exp,
                             bias=neg1[:, :], scale=1.0 / TAU)
        mask = spool.tile([P, max(TMC, 1)], F32, tag="mask")
        nc.vector.tensor_scalar(mask[:], times_sb[:], TIME, None, alu.is_le)
        w_all = cpool.tile([P, max(TMC, 1)], F32, tag="w")
        nc.vector.tensor_tensor(w_all[:], e_all[:], mask[:], alu.mult)
        w16 = cpool.tile([P, max(TMC, 1)], F16, tag="w16")
        nc.vector.tensor_scalar(w16[:], w_all[:], 0.0, None, alu.add)

        total = spool.tile([P, Tn], F32, tag="total")
        toff = 0
        for (t0, S, Ks) in slabs:
            if Ks == 0:
                nc.vector.memset(total[:, t0:t0 + S], 0.0)
                continue
            nc.vector.tensor_reduce(
                total[:, t0:t0 + S],
                w_all[:, toff:toff + S * Ks].rearrange(
                    "p (s k) -> p s k", k=Ks),
                axis=mybir.AxisListType.X, op=alu.add)
            toff += S * Ks
        iszero = spool.tile([P, Tn], F32, tag="iszero")
        nc.vector.tensor_scalar(iszero[:], total[:], 0.0, None, alu.is_equal)
        total_adj = spool.tile([P, Tn], F32, tag="tadj")
        nc.vector.tensor_tensor(total_adj[:], total[:], iszero[:], alu.add)
        inv_total = spool.tile([P, Tn], F32, tag="invt")
        nc.vector.reciprocal(inv_total[:], total_adj[:])
        # denom = 1 + sum_k w_k/total is exactly 2 (total>0) or 1 (total==0)
        inv_denom = cpool.tile([P, Tn], F32, tag="invd")
        nc.vector.tensor_scalar(inv_denom[:], iszero[:], 0.5, 0.5,
                                alu.mult, alu.add)
        c1 = cpool.tile([P, Tn], F32, tag="c1")
        nc.vector.tensor_tensor(c1[:], inv_total[:], inv_denom[:], alu.mult)

        with tc.tile_pool(name="psum", bufs=4, space="PSUM") as ppool:
            ioff = 0
            toff = 0
            for (t0, S, Ks) in slabs:
                KQ = Ks + 1
                G = gpool.tile([P, maxs * maxg * D], F16, tag="G")
                for s in range(S):
                    for q in range(KQ):
                        col = ioff + s * KQ + q
                        nc.gpsimd.indirect_dma_start(
                            G[:, (s * KQ + q) * D:(s * KQ + q + 1) * D],
                            None,
                            feats[:, :],
                            IndirectOffsetOnAxis(
                                ap=idx_sb[:, col:col + 1], axis=0),
                        )

                if Ks > 0:
                    a_ks = min(act_ks, Ks)
                    wG = wgpool.tile([P, maxs * (maxg - 1) * D], F16,
                                     tag="wG")
                    for s in range(S):
                        woff = toff + s * Ks
                        for k in range(a_ks):
                            nc.scalar.activation(
                                wG[:, (s * Ks + k) * D:(s * Ks + k + 1) * D],
                                G[:, (s * KQ + 1 + k) * D:
                                  (s * KQ + 2 + k) * D],
                                act.Copy,
                                scale=w_all[:, woff + k:woff + k + 1])
                        if Ks > a_ks:
                            nc.vector.tensor_tensor(
                                wG[:, (s * Ks + a_ks) * D:(s + 1) * Ks * D]
                                .rearrange("p (k d) -> p k d", d=D),
                                G[:, (s * KQ + 1 + a_ks) * D:(s + 1) * KQ * D]
                                .rearrange("p (k d) -> p k d", d=D),
                                w16[:, woff + a_ks:woff + Ks]
                                .to_broadcast((P, Ks - a_ks, D)),
                                alu.mult)

                    psum = ppool.tile([P, SLAB * D], F32, tag="ps")
                    wG_v = wG[:, :S * Ks * D].rearrange(
                        "p (s k d) -> p s k d", k=Ks, d=D)
                    ps_v = psum[:, :S * D].rearrange("p (s d) -> p s d", d=D)
                    for k in range(Ks):
                        nc.tensor.matmul(
                            ps_v, ident[:], wG_v[:, :, k:k + 1, :],
                            start=(k == 0), stop=(k == Ks - 1))

                for s in range(S):
                    t = t0 + s
                    ot = fpool.tile([P, D], F32, tag="ot")
                    if Ks > 0:
                        tmp = fpool.tile([P, D], F32, tag="tmp")
                        nc.vector.tensor_scalar(
                            tmp[:], psum[:, s * D:(s + 1) * D],
                            c1[:, t:t + 1], None, alu.mult)
                        nc.vector.scalar_tensor_tensor(
                            ot[:], G[:, s * KQ * D:(s * KQ + 1) * D],
                            inv_denom[:, t:t + 1], tmp[:],
                            op0=alu.mult, op1=alu.add)
                    else:
                        nc.vector.tensor_scalar(
                            ot[:], G[:, s * KQ * D:(s * KQ + 1) * D],
                            inv_denom[:, t:t + 1], None, alu.mult)
                    nc.sync.dma_start(out[t * P:(t + 1) * P, :], ot[:])
                ioff += S * KQ
                toff += S * Ks


_NC_CACHE = {}


def _get_nc(slabs):
    key = tuple(slabs)
    if key not in _NC_CACHE:
        TMC = sum(s * ks for _, s, ks in slabs)
        ICOLS = sum(s * (ks + 1) for _, s, ks in slabs)
        nc = bacc.Bacc("TRN2", target_bir_lowering=False, debug=False,
                       enable_asserts=False,
                       dynamic_dma_scratch_size=49152)
        feats = nc.dram_tensor("features", [N, D], F16,
                               kind="ExternalInput").ap()
        idx = nc.dram_tensor("idx", [P, ICOLS], I32,
                             kind="ExternalInput").ap()
        identd = nc.dram_tensor("ident", [P, P], F16,
                                kind="ExternalInput").ap()
        times = nc.dram_tensor("times", [P, max(TMC, 1)], F32,
                               kind="ExternalInput").ap()
        out = nc.dram_tensor("out", [B_PAD, D], F32,
                             kind="ExternalOutput").ap()
        with tile.TileContext(nc) as tc:
            _build_kernel(tc, {"out": out},
                          {"features": feats, "idx": idx, "times": times,
                           "ident": identd},
                          slabs)
        nc.compile()
        _NC_CACHE[key] = nc
    return _NC_CACHE[key]


def kernel(nodes, neigh_ids, neigh_times, features):
    global LAST_RESULT
    nodes = np.asarray(nodes).astype(np.int32, copy=False)
    neigh_ids = np.asarray(neigh_ids).astype(np.int32, copy=False)
    neigh_times = np.asarray(neigh_times).astype(np.float32, copy=False)
    features16 = np.ascontiguousarray(
        np.asarray(features, dtype=np.float32)).astype(np.float16)

    # compact valid neighbors (t <= TIME) to the front; dropped slots have
    # weight exactly 0 in the reference, so results are identical
    valid = neigh_times <= TIME
    cnt = valid.sum(1).astype(np.int32)
    ordc = np.argsort(~valid, axis=1, kind='stable')
    nid_c = np.take_along_axis(neigh_ids, ordc, 1)
    nt_c = np.take_along_axis(neigh_times, ordc, 1)
    pos = np.arange(K)[None, :]
    nid_c = np.where(pos < cnt[:, None], nid_c, 0).astype(np.int32)
    nt_c = np.where(pos < cnt[:, None], nt_c, 200.0).astype(np.float32)

    # sort nodes by valid count and deal round-robin so all 8 cores share
    # nearly identical per-tile slot counts (one compiled kernel for all)
    order = np.argsort(cnt, kind='stable')
    core_rows = [order[c::N_CORES] for c in range(N_CORES)]

    nodes_cs, nbr_cs, ts_cs, cnt_cs = [], [], [], []
    for c in range(N_CORES):
        rows = core_rows[c]
        nodes_c = np.zeros(B_PAD, np.int32)
        nodes_c[PADROWS:] = nodes[rows]
        nbr = np.zeros((B_PAD, K), np.int32)
        nbr[PADROWS:] = nid_c[rows]
        ts = np.full((B_PAD, K), 200.0, np.float32)
        ts[PADROWS:] = nt_c[rows]
        cp = np.zeros(B_PAD, np.int32)
        cp[PADROWS:] = cnt[rows]
        nodes_cs.append(nodes_c)
        nbr_cs.append(nbr)
        ts_cs.append(ts)
        cnt_cs.append(cp)

    tile_max = np.stack(
        [cp.reshape(T, P).max(1) for cp in cnt_cs]).max(0)  # [T]
    slabs = []
    t0 = 0
    while t0 < T:
        S = min(SLAB, T - t0)
        Ks = int(tile_max[t0:t0 + S].max())
        slabs.append((t0, S, Ks))
        t0 += S

    in_maps = []
    for c in range(N_CORES):
        idx_parts = []
        tm_parts = []
        for (t0, S, Ks) in slabs:
            r0, r1 = t0 * P, (t0 + S) * P
            blk = np.concatenate(
                [nodes_cs[c][r0:r1, None], nbr_cs[c][r0:r1, :Ks]], axis=1)
            idx_parts.append(
                blk.reshape(S, P, Ks + 1).transpose(1, 0, 2).reshape(P, -1))
            if Ks > 0:
                tb = ts_cs[c][r0:r1, :Ks]
                tm_parts.append(
                    tb.reshape(S, P, Ks).transpose(1, 0, 2).reshape(P, -1))
        idx_t = np.ascontiguousarray(np.concatenate(idx_parts, axis=1))
        times_t = (np.ascontiguousarray(np.concatenate(tm_parts, axis=1))
                   if tm_parts else np.zeros((P, 1), np.float32))
        in_maps.append(
            {"features": features16, "idx": idx_t, "times": times_t,
             "ident": np.eye(P, dtype=np.float16)})

    nc = _get_nc(slabs)
    res = run_bass_kernel_spmd(nc, in_maps, core_ids=list(range(N_CORES)))
    LAST_RESULT = res
    out = np.empty((B, D), np.float32)
    for c in range(N_CORES):
        out[core_rows[c]] = res.results[c]["out"][PADROWS:]
    return out
